# revision 1
# baseline (speedup 1.0000x reference)
"""Trainium2 Bass kernel for batched tanh-query attention.

Per-batch computation (B=8, one batch per NeuronCore, pure data parallel):
    q = tanh(out_state)            [Q, H]    Q=K=2048, H=128
    S = q @ history.T              [Q, K]
    P = softmax(S, axis=K)
    attn = P @ history             [Q, H]

Implementation is flash-style (no HBM intermediates), computed in the
"transposed" orientation S_T[k, q] so that the second matmul needs no
transpose of P. Softmax denominator comes from an all-ones stationary
matmul (d broadcast over partitions), un-normalized attn_T is accumulated
in PSUM over key blocks, and the epilogue transposes back to q-major and
applies 1/d.
"""

import sys

for _p in ("/opt/trn_rl_repo", "/opt/trn_rl_repo/concourse"):
    if _p not in sys.path:
        sys.path.insert(0, _p)

import numpy as np

N_CORES = 8
SEQ = 2048
H = 128
P = 128
T = SEQ // P          # 16 seq tiles
NHALF = 2             # process queries in 2 halves of 1024 (PSUM budget)
QH = SEQ // NHALF     # 1024
QTPH = QH // P        # 8 q-tiles per half

_CACHE = {}


def _build():
    from concourse import bacc, bass, masks, mybir, tile

    f32 = mybir.dt.float32
    bf16 = mybir.dt.bfloat16
    AF = mybir.ActivationFunctionType

    nc = bacc.Bacc("TRN2", target_bir_lowering=False, debug=False,
                   num_devices=N_CORES)
    os_d = nc.dram_tensor("out_state", (SEQ, H), f32, kind="ExternalInput")
    h_d = nc.dram_tensor("history", (SEQ, H), f32, kind="ExternalInput")
    a_d = nc.dram_tensor("attn", (SEQ, H), f32, kind="ExternalOutput")

    with tile.TileContext(nc) as tc:
        with (
            tc.tile_pool(name="const", bufs=1) as constp,
            tc.tile_pool(name="big", bufs=1) as bigp,
            tc.tile_pool(name="stage", bufs=2) as stagep,
            tc.tile_pool(name="work", bufs=3) as workp,
            tc.tile_pool(name="ps", bufs=2, space=bass.MemorySpace.PSUM) as psp,
            tc.tile_pool(name="psacc", bufs=1, space=bass.MemorySpace.PSUM) as pacc,
        ):
            id_bf = constp.tile([P, P], bf16, tag="idbf")
            masks.make_identity(nc, id_bf[:])
            id_f32 = constp.tile([P, P], f32, tag="idf")
            masks.make_identity(nc, id_f32[:])
            ones_bf = constp.tile([P, P], bf16, tag="ones")
            nc.vector.memset(ones_bf[:], 1.0)

            # persistent bf16 operands
            hn = bigp.tile([P, T, P], bf16, tag="hn")    # [k_in, t, h] natural
            ht = bigp.tile([P, T, P], bf16, tag="ht")    # [h, t, k_in] transposed
            qT = bigp.tile([P, T, P], bf16, tag="qT")    # [h, t, q_in] transposed

            # ---- load + preprocess ----
            os_f = stagep.tile([P, T, H], f32, tag="ldin")
            nc.sync.dma_start(os_f[:], os_d[:].rearrange("(t p) h -> p t h", p=P))
            q_nat = stagep.tile([P, T, H], bf16, tag="qnat")
            nc.scalar.activation(q_nat[:], os_f[:], AF.Tanh)

            hn_f = stagep.tile([P, T, H], f32, tag="ldin")
            nc.sync.dma_start(hn_f[:], h_d[:].rearrange("(t p) h -> p t h", p=P))
            nc.vector.tensor_copy(hn[:], hn_f[:])

            for t in range(T):
                tp = psp.tile([P, P], bf16, tag="st")
                nc.tensor.transpose(tp[:], q_nat[:, t, :], id_bf[:])
                nc.vector.tensor_copy(qT[:, t, :], tp[:])
            for t in range(T):
                tp = psp.tile([P, P], bf16, tag="st")
                nc.tensor.transpose(tp[:], hn[:, t, :], id_bf[:])
                nc.vector.tensor_copy(ht[:, t, :], tp[:])

            # ---- main flash loop ----
            for qh in range(NHALF):
                attnT = pacc.tile([P, QH], f32, tag="acc")   # [h, q_local]
                dbc = pacc.tile([P, QH], f32, tag="dbc")     # d broadcast rows
                for kb in range(T):
                    st = psp.tile([P, QH], f32, tag="st")    # S_T[k_in, q_local]
                    for c in range(2):
                        rhs = qT[:, qh * QTPH + 4 * c: qh * QTPH + 4 * (c + 1), :]
                        nc.tensor.matmul(st[:, 512 * c: 512 * (c + 1)],
                                         ht[:, kb, :], rhs,
                                         start=True, stop=True)
                    ex = workp.tile([P, QH], bf16, tag="ex")
                    nc.scalar.activation(ex[:], st[:], AF.Exp)
                    first = kb == 0
                    last = kb == T - 1
                    for c in range(2):
                        exc = ex[:, 512 * c: 512 * (c + 1)]
                        nc.tensor.matmul(attnT[:, 512 * c: 512 * (c + 1)],
                                         hn[:, kb, :], exc,
                                         start=first, stop=last)
                        nc.tensor.matmul(dbc[:, 512 * c: 512 * (c + 1)],
                                         ones_bf[:], exc,
                                         start=first, stop=last)

                # ---- epilogue for this half ----
                aT_sb = workp.tile([P, QH], f32, tag="atsb")
                nc.vector.tensor_copy(aT_sb[:], attnT[:])
                d_sb = workp.tile([P, QH], f32, tag="dsb")
                nc.vector.tensor_copy(d_sb[:], dbc[:])
                for t in range(QTPH):
                    dps = psp.tile([P, P], f32, tag="st")
                    nc.tensor.transpose(dps[:], d_sb[:, P * t: P * (t + 1)],
                                        id_f32[:])
                    rc = workp.tile([P, 1], f32, tag="rc")
                    nc.vector.reciprocal(rc[:], dps[:, 0:1])
                    aps = psp.tile([P, P], f32, tag="st")
                    nc.tensor.transpose(aps[:], aT_sb[:, P * t: P * (t + 1)],
                                        id_f32[:])
                    ot = workp.tile([P, P], f32, tag="ot")
                    nc.vector.tensor_scalar_mul(ot[:], aps[:], rc[:])
                    row0 = qh * QH + P * t
                    nc.sync.dma_start(a_d[row0: row0 + P, :], ot[:])

    nc.compile()
    return nc


def _get_nc():
    if "nc" not in _CACHE:
        _CACHE["nc"] = _build()
    return _CACHE["nc"]


def _run(out_state, history, trace=False):
    from concourse.bass_utils import run_bass_kernel_spmd

    nc = _get_nc()
    out_state = np.ascontiguousarray(out_state, dtype=np.float32)
    history = np.ascontiguousarray(history, dtype=np.float32)
    in_maps = [
        {"out_state": out_state[b], "history": history[b]}
        for b in range(N_CORES)
    ]
    res = run_bass_kernel_spmd(nc, in_maps, core_ids=list(range(N_CORES)),
                               trace=trace)
    attn = np.stack([res.results[b]["attn"] for b in range(N_CORES)], axis=0)
    return attn.astype(np.float32), res


def kernel(out_state, history):
    attn, _ = _run(out_state, history)
    return attn


# revision 3
# speedup vs baseline: 1.0554x; 1.0554x over previous
"""Trainium2 Bass kernel for batched tanh-query attention.

Per-batch computation (B=8, one batch per NeuronCore, pure data parallel):
    q = tanh(out_state)            [Q, H]    Q=K=2048, H=128
    S = q @ history.T              [Q, K]
    P = softmax(S, axis=K)
    attn = P @ history             [Q, H]

Flash-style, no HBM intermediates, computed in the transposed orientation
S_T[k, q] so the second matmul needs no transpose of P:
  MM1:  S_T[kb]   = ht[kb].T @ qT          (PE, fp32 PSUM, 512-wide chunks)
  exp:  E[kb]     = exp(S_T[kb])           (ACT, PSUM -> SBUF bf16)
  MM2:  attn_T   += hn[kb].T @ E[kb]       (PE, accumulate over kb)
  d:    pairwise-add tree over E[kb] (DVE bf16) then ones.T @ partials (PE)
  epilogue: PE-transpose attn_T and d back to q-major, multiply by 1/d,
  DMA out. Input operands are bf16; transposed layouts come from the DMA
  xbar transpose.
"""

import sys

for _p in ("/opt/trn_rl_repo", "/opt/trn_rl_repo/concourse"):
    if _p not in sys.path:
        sys.path.insert(0, _p)

import numpy as np

N_CORES = 8
SEQ = 2048
H = 128
P = 128
T = SEQ // P          # 16 seq tiles
NHALF = 2             # queries processed in 2 halves of 1024 (PSUM budget)
QH = SEQ // NHALF     # 1024
QTPH = QH // P        # 8 q-tiles per half
NC = 2                # 512-wide chunks per half
CW = QH // NC         # 512

_CACHE = {}


def _build():
    from concourse import bacc, bass, masks, mybir, tile

    f32 = mybir.dt.float32
    bf16 = mybir.dt.bfloat16
    AF = mybir.ActivationFunctionType

    nc = bacc.Bacc("TRN2", target_bir_lowering=False, debug=False,
                   num_devices=N_CORES)
    os_d = nc.dram_tensor("out_state", (SEQ, H), f32, kind="ExternalInput")
    h_d = nc.dram_tensor("history", (SEQ, H), f32, kind="ExternalInput")
    a_d = nc.dram_tensor("attn", (SEQ, H), f32, kind="ExternalOutput")

    with tile.TileContext(nc) as tc:
        with (
            tc.tile_pool(name="const", bufs=1) as constp,
            tc.tile_pool(name="big", bufs=1) as bigp,
            tc.tile_pool(name="stage", bufs=2) as stagep,
            tc.tile_pool(name="work", bufs=3) as workp,
            tc.tile_pool(name="expool", bufs=6) as expool,
            tc.tile_pool(name="dtree", bufs=8) as dtreep,
            tc.tile_pool(name="ps", bufs=3, space=bass.MemorySpace.PSUM) as psp,
            tc.tile_pool(name="psacc", bufs=1, space=bass.MemorySpace.PSUM) as pacc,
            tc.tile_pool(name="psd", bufs=2, space=bass.MemorySpace.PSUM) as psd,
        ):
            id_f32 = constp.tile([P, P], f32, tag="idf")
            masks.make_identity(nc, id_f32[:])
            ones_bf = constp.tile([P, P], bf16, tag="ones")
            nc.vector.memset(ones_bf[:], 1.0)

            # persistent bf16 operands
            hn = bigp.tile([P, T, P], bf16, tag="hn")    # [k_in, t, h] natural
            ht = bigp.tile([P, T, P], bf16, tag="ht")    # [h, t, k_in] transposed
            qT = bigp.tile([P, T, P], bf16, tag="qT")    # [h, t, q_in] transposed

            # ---- load + preprocess ----
            os_f = stagep.tile([P, T, H], f32, tag="ldin")
            nc.sync.dma_start(os_f[:], os_d[:].rearrange("(t p) h -> p t h", p=P))
            q_nat = stagep.tile([P, T, H], bf16, tag="qnat")
            for j in range(4):
                nc.scalar.activation(q_nat[:, 4 * j: 4 * (j + 1), :],
                                     os_f[:, 4 * j: 4 * (j + 1), :], AF.Tanh)

            hn_f = stagep.tile([P, T, H], f32, tag="ldin")
            nc.sync.dma_start(hn_f[:], h_d[:].rearrange("(t p) h -> p t h", p=P))
            nc.vector.tensor_copy(hn[:], hn_f[:])

            # transposed layouts via DMA xbar (bf16 SBUF->SBUF)
            for t in range(QTPH):                      # qT for half 0 first
                nc.sync.dma_start_transpose(qT[:, t, :], q_nat[:, t, :])
            for t in range(T):
                nc.sync.dma_start_transpose(ht[:, t, :], hn[:, t, :])
            for t in range(QTPH, T):                   # qT for half 1
                nc.sync.dma_start_transpose(qT[:, t, :], q_nat[:, t, :])

            # ---- main flash loop ----
            for qh in range(NHALF):
                attnT = pacc.tile([P, QH], f32, tag="acc")   # [h, q_local]
                # per-chunk d accumulators (PSUM) and bf16 add-trees
                dbc = [psd.tile([P, CW], f32, tag="dbc", name=f"dbc{qh}_{i}")
                       for i in range(NC)]
                lvl1 = [[] for _ in range(NC)]
                lvl2 = [[] for _ in range(NC)]
                exprev = [None] * NC
                for kb in range(T):
                    first = kb == 0
                    last = kb == T - 1
                    for c in range(2):
                        st = psp.tile([P, CW], f32, tag="st")
                        rhs = qT[:, qh * QTPH + 4 * c: qh * QTPH + 4 * (c + 1), :]
                        nc.tensor.matmul(st[:], ht[:, kb, :], rhs,
                                         start=True, stop=True)
                        ex = expool.tile([P, CW], bf16, tag="ex")
                        nc.scalar.activation(ex[:], st[:], AF.Exp)
                        nc.tensor.matmul(attnT[:, CW * c: CW * (c + 1)],
                                         hn[:, kb, :], ex[:],
                                         start=first, stop=last)
                        # d tree: pair up exp tiles on DVE (bf16 adds)
                        if kb % 2 == 0:
                            exprev[c] = ex
                        else:
                            t1 = dtreep.tile([P, CW], bf16, tag="l1")
                            nc.vector.tensor_add(t1[:], exprev[c][:], ex[:])
                            lvl1[c].append(t1)
                            exprev[c] = None
                            if len(lvl1[c]) % 2 == 0:
                                a, b = lvl1[c][-2], lvl1[c][-1]
                                t2 = dtreep.tile([P, CW], bf16, tag="l2")
                                nc.vector.tensor_add(t2[:], a[:], b[:])
                                lvl2[c].append(t2)
                # ones-matmul over the 4 level-2 partials -> d broadcast
                for c in range(NC):
                    assert len(lvl2[c]) == 4
                    for i, pt in enumerate(lvl2[c]):
                        nc.tensor.matmul(dbc[c][:], ones_bf[:], pt[:],
                                         start=(i == 0), stop=(i == 3))

                # ---- epilogue for this half ----
                aT_sb = workp.tile([P, QH], f32, tag="atsb")
                nc.vector.tensor_copy(aT_sb[:], attnT[:])
                d_sb = workp.tile([P, QH], f32, tag="dsb")
                for c in range(NC):
                    nc.vector.tensor_copy(d_sb[:, CW * c: CW * (c + 1)], dbc[c][:])
                for t in range(QTPH):
                    dps = psp.tile([P, P], f32, tag="st")
                    nc.tensor.transpose(dps[:], d_sb[:, P * t: P * (t + 1)],
                                        id_f32[:])
                    rc = workp.tile([P, 1], f32, tag="rc")
                    nc.vector.reciprocal(rc[:], dps[:, 0:1])
                    aps = psp.tile([P, P], f32, tag="st")
                    nc.tensor.transpose(aps[:], aT_sb[:, P * t: P * (t + 1)],
                                        id_f32[:])
                    ot = workp.tile([P, P], f32, tag="ot")
                    nc.vector.tensor_scalar_mul(ot[:], aps[:], rc[:])
                    row0 = qh * QH + P * t
                    nc.sync.dma_start(a_d[row0: row0 + P, :], ot[:])

    nc.compile()
    return nc


def _get_nc():
    if "nc" not in _CACHE:
        _CACHE["nc"] = _build()
    return _CACHE["nc"]


def _run(out_state, history, trace=False):
    from concourse.bass_utils import run_bass_kernel_spmd

    nc = _get_nc()
    out_state = np.ascontiguousarray(out_state, dtype=np.float32)
    history = np.ascontiguousarray(history, dtype=np.float32)
    in_maps = [
        {"out_state": out_state[b], "history": history[b]}
        for b in range(N_CORES)
    ]
    res = run_bass_kernel_spmd(nc, in_maps, core_ids=list(range(N_CORES)),
                               trace=trace)
    attn = np.stack([res.results[b]["attn"] for b in range(N_CORES)], axis=0)
    return attn.astype(np.float32), res


def kernel(out_state, history):
    attn, _ = _run(out_state, history)
    return attn


# revision 6
# speedup vs baseline: 1.0906x; 1.0333x over previous
"""Trainium2 Bass kernel for batched tanh-query attention.

Per-batch computation (B=8, one batch per NeuronCore, pure data parallel):
    q = tanh(out_state)            [Q, H]    Q=K=2048, H=128
    S = q @ history.T              [Q, K]
    P = softmax(S, axis=K)
    attn = P @ history             [Q, H]

Flash-style, no HBM intermediates, computed in the transposed orientation
S_T[k, q] so the second matmul needs no transpose of P:
  MM1:  S_T[kb]   = ht[kb].T @ qT          (PE, fp32 PSUM, 512-wide chunks)
  exp:  E[kb]     = exp(S_T[kb])           (ACT, PSUM -> SBUF bf16)
  MM2:  attn_T   += hn[kb].T @ E[kb]       (PE, accumulate over kb)
  d:    pairwise-add tree over E[kb] (DVE bf16) then ones.T @ partials (PE)
  epilogue: PE-transpose attn_T and d back to q-major, multiply by 1/d,
  DMA out. Input operands are bf16; transposed layouts come from the DMA
  xbar transpose.
"""

import sys

for _p in ("/opt/trn_rl_repo", "/opt/trn_rl_repo/concourse"):
    if _p not in sys.path:
        sys.path.insert(0, _p)

import numpy as np

N_CORES = 8
SEQ = 2048
H = 128
P = 128
T = SEQ // P          # 16 seq tiles
NHALF = 2             # queries processed in 2 halves of 1024 (PSUM budget)
QH = SEQ // NHALF     # 1024
QTPH = QH // P        # 8 q-tiles per half
NC = 2                # 512-wide chunks per half
CW = QH // NC         # 512

_CACHE = {}


def _build():
    from concourse import bacc, bass, masks, mybir, tile

    f32 = mybir.dt.float32
    bf16 = mybir.dt.bfloat16
    AF = mybir.ActivationFunctionType

    nc = bacc.Bacc("TRN2", target_bir_lowering=False, debug=False,
                   num_devices=N_CORES)
    os_d = nc.dram_tensor("out_state", (SEQ, H), f32, kind="ExternalInput")
    h_d = nc.dram_tensor("history", (SEQ, H), f32, kind="ExternalInput")
    a_d = nc.dram_tensor("attn", (SEQ, H), f32, kind="ExternalOutput")

    with tile.TileContext(nc) as tc:
        with (
            tc.tile_pool(name="const", bufs=1) as constp,
            tc.tile_pool(name="big", bufs=1) as bigp,
            tc.tile_pool(name="stage", bufs=2) as stagep,
            tc.tile_pool(name="work", bufs=3) as workp,
            tc.tile_pool(name="expool", bufs=6) as expool,
            tc.tile_pool(name="dtree", bufs=4) as dtreep,
            tc.tile_pool(name="ps", bufs=4, space=bass.MemorySpace.PSUM) as psp,
            tc.tile_pool(name="psacc", bufs=1, space=bass.MemorySpace.PSUM) as pacc,
            tc.tile_pool(name="psd", bufs=2, space=bass.MemorySpace.PSUM) as psd,
        ):
            id_f32 = constp.tile([P, P], f32, tag="idf")
            masks.make_identity(nc, id_f32[:])
            ones_bf = constp.tile([P, P], bf16, tag="ones")
            nc.vector.memset(ones_bf[:], 1.0)

            # persistent bf16 operands
            hn = bigp.tile([P, T, P], bf16, tag="hn")    # [k_in, t, h] natural
            ht = bigp.tile([P, T, P], bf16, tag="ht")    # [h, t, k_in] transposed
            qT = bigp.tile([P, T, P], bf16, tag="qT")    # [h, t, q_in] transposed

            # ---- load + preprocess ----
            os_f = stagep.tile([P, T, H], f32, tag="ldin")
            nc.sync.dma_start(os_f[:], os_d[:].rearrange("(t p) h -> p t h", p=P))
            hn_f = stagep.tile([P, T, H], f32, tag="ldin")
            nc.sync.dma_start(hn_f[:], h_d[:].rearrange("(t p) h -> p t h", p=P))

            q_nat = stagep.tile([P, T, H], bf16, tag="qnat")
            nc.scalar.activation(q_nat[:, 0:4, :], os_f[:, 0:4, :], AF.Tanh)
            nc.vector.tensor_copy(hn[:], hn_f[:])

            # Transposed layouts via DMA xbar (bf16 SBUF->SBUF), alternating
            # the two HWDGE queues (sync / scalar). Order matters: the first
            # MM1 needs qT tiles 0-3 and ht[0], so those go first.
            xq = [nc.sync, nc.scalar]
            seq = [("q", t) for t in range(4)] + [("h", 0), ("h", 1)]
            for j in range(1, 4):
                nc.scalar.activation(q_nat[:, 4 * j: 4 * (j + 1), :],
                                     os_f[:, 4 * j: 4 * (j + 1), :], AF.Tanh)
            seq += [("q", t) for t in range(4, QTPH)]
            seq += [("h", t) for t in range(2, T)]
            seq += [("q", t) for t in range(QTPH, T)]
            for i, (kind, t) in enumerate(seq):
                eng = xq[i % 2]
                if kind == "q":
                    eng.dma_start_transpose(qT[:, t, :], q_nat[:, t, :])
                else:
                    eng.dma_start_transpose(ht[:, t, :], hn[:, t, :])

            # ---- main flash loop ----
            for qh in range(NHALF):
                attnT = pacc.tile([P, QH], f32, tag="acc")   # [h, q_local]
                # per-chunk d accumulators (PSUM)
                dbc = [psd.tile([P, CW], f32, tag="dbc", name=f"dbc{qh}_{i}")
                       for i in range(NC)]
                exprev = [None] * NC
                for kb in range(T):
                    first = kb == 0
                    last = kb == T - 1
                    for c in range(2):
                        st = psp.tile([P, CW], f32, tag="st")
                        rhs = qT[:, qh * QTPH + 4 * c: qh * QTPH + 4 * (c + 1), :]
                        nc.tensor.matmul(st[:], ht[:, kb, :], rhs,
                                         start=True, stop=True)
                        ex = expool.tile([P, CW], bf16, tag="ex")
                        nc.scalar.activation(ex[:], st[:], AF.Exp)
                        nc.tensor.matmul(attnT[:, CW * c: CW * (c + 1)],
                                         hn[:, kb, :], ex[:],
                                         start=first, stop=last)
                        # d: pair-add exp tiles on DVE (bf16), then
                        # accumulate pair sums via ones-matmul in PSUM
                        if kb % 2 == 0:
                            exprev[c] = ex
                        else:
                            t1 = dtreep.tile([P, CW], bf16, tag="l1")
                            nc.vector.tensor_add(t1[:], exprev[c][:], ex[:])
                            exprev[c] = None
                            nc.tensor.matmul(dbc[c][:], ones_bf[:], t1[:],
                                             start=(kb == 1), stop=last)

                # ---- epilogue for this half ----
                aT_sb = workp.tile([P, QH], f32, tag="atsb")
                nc.vector.tensor_copy(aT_sb[:], attnT[:])
                d_sb = workp.tile([P, QH], f32, tag="dsb")
                for c in range(NC):
                    nc.vector.tensor_copy(d_sb[:, CW * c: CW * (c + 1)], dbc[c][:])
                for t in range(QTPH):
                    dps = psp.tile([P, P], f32, tag="st")
                    nc.tensor.transpose(dps[:], d_sb[:, P * t: P * (t + 1)],
                                        id_f32[:])
                    rc = workp.tile([P, 1], f32, tag="rc")
                    nc.vector.reciprocal(rc[:], dps[:, 0:1])
                    aps = psp.tile([P, P], f32, tag="st")
                    nc.tensor.transpose(aps[:], aT_sb[:, P * t: P * (t + 1)],
                                        id_f32[:])
                    ot = workp.tile([P, P], f32, tag="ot")
                    nc.vector.tensor_scalar_mul(ot[:], aps[:], rc[:])
                    row0 = qh * QH + P * t
                    nc.sync.dma_start(a_d[row0: row0 + P, :], ot[:])

    nc.compile()
    return nc


def _get_nc():
    if "nc" not in _CACHE:
        _CACHE["nc"] = _build()
    return _CACHE["nc"]


def _run(out_state, history, trace=False):
    from concourse.bass_utils import run_bass_kernel_spmd

    nc = _get_nc()
    out_state = np.ascontiguousarray(out_state, dtype=np.float32)
    history = np.ascontiguousarray(history, dtype=np.float32)
    in_maps = [
        {"out_state": out_state[b], "history": history[b]}
        for b in range(N_CORES)
    ]
    res = run_bass_kernel_spmd(nc, in_maps, core_ids=list(range(N_CORES)),
                               trace=trace)
    attn = np.stack([res.results[b]["attn"] for b in range(N_CORES)], axis=0)
    return attn.astype(np.float32), res


def kernel(out_state, history):
    attn, _ = _run(out_state, history)
    return attn


# revision 9
# speedup vs baseline: 1.3135x; 1.2044x over previous
"""Trainium2 Bass kernel for batched tanh-query attention.

Per-batch computation (B=8, one batch per NeuronCore, pure data parallel):
    q = tanh(out_state)            [Q, H]    Q=K=2048, H=128
    S = q @ history.T              [Q, K]
    P = softmax(S, axis=K)
    attn = P @ history             [Q, H]

Flash-style, no HBM intermediates, computed in the transposed orientation
S_T[k, q] so the second matmul needs no transpose of P:
  MM1:  S_T[kb]   = ht[kb].T @ qT          (PE, fp32 PSUM, 512-wide chunks)
  exp:  E[kb]     = exp(S_T[kb])           (ACT, PSUM -> SBUF bf16)
  MM2:  attn_T   += hn[kb].T @ E[kb]       (PE, accumulate over kb)
  d:    pairwise-add tree over E[kb] (DVE bf16) then ones.T @ partials (PE)
  epilogue: PE-transpose attn_T and d back to q-major, multiply by 1/d,
  DMA out. Input operands are bf16; transposed layouts come from the DMA
  xbar transpose.
"""

import sys

for _p in ("/opt/trn_rl_repo", "/opt/trn_rl_repo/concourse"):
    if _p not in sys.path:
        sys.path.insert(0, _p)

import numpy as np

N_CORES = 8
SEQ = 2048
H = 128
P = 128
T = SEQ // P          # 16 seq tiles
NHALF = 2             # queries processed in 2 halves of 1024 (PSUM budget)
QH = SEQ // NHALF     # 1024
QTPH = QH // P        # 8 q-tiles per half
NC = 2                # 512-wide chunks per half
CW = QH // NC         # 512

_CACHE = {}


def _build():
    from concourse import bacc, bass, masks, mybir, tile

    f32 = mybir.dt.float32
    bf16 = mybir.dt.bfloat16
    AF = mybir.ActivationFunctionType

    nc = bacc.Bacc("TRN2", target_bir_lowering=False, debug=False,
                   num_devices=N_CORES)
    os_d = nc.dram_tensor("out_state", (SEQ, H), f32, kind="ExternalInput")
    h_d = nc.dram_tensor("history", (SEQ, H), f32, kind="ExternalInput")
    a_d = nc.dram_tensor("attn", (SEQ, H), f32, kind="ExternalOutput")

    with tile.TileContext(nc) as tc:
        with (
            tc.tile_pool(name="const", bufs=1) as constp,
            tc.tile_pool(name="big", bufs=1) as bigp,
            tc.tile_pool(name="stage", bufs=2) as stagep,
            tc.tile_pool(name="work", bufs=3) as workp,
            tc.tile_pool(name="expool", bufs=6) as expool,
            tc.tile_pool(name="dtree", bufs=4) as dtreep,
            tc.tile_pool(name="ps", bufs=4, space=bass.MemorySpace.PSUM) as psp,
            tc.tile_pool(name="psacc", bufs=1, space=bass.MemorySpace.PSUM) as pacc,
            tc.tile_pool(name="psd", bufs=2, space=bass.MemorySpace.PSUM) as psd,
        ):
            id_f32 = constp.tile([P, P], f32, tag="idf")
            masks.make_identity(nc, id_f32[:])
            id_bf = constp.tile([P, P], bf16, tag="idb")
            masks.make_identity(nc, id_bf[:])
            ones_bf = constp.tile([P, P], bf16, tag="ones")
            nc.vector.memset(ones_bf[:], 1.0)

            # persistent bf16 operands
            hn = bigp.tile([P, T, P], bf16, tag="hn")    # [k_in, t, h] natural
            ht = bigp.tile([P, T, P], bf16, tag="ht")    # [h, t, k_in] transposed
            qT = bigp.tile([P, T, P], bf16, tag="qT")    # [h, t, q_in] transposed

            # ---- load + preprocess ----
            os_f = stagep.tile([P, T, H], f32, tag="ldin")
            nc.sync.dma_start(os_f[:], os_d[:].rearrange("(t p) h -> p t h", p=P))
            hn_f = stagep.tile([P, T, H], f32, tag="ldin")
            nc.sync.dma_start(hn_f[:], h_d[:].rearrange("(t p) h -> p t h", p=P))

            q_nat = stagep.tile([P, T, H], bf16, tag="qnat")
            for j in range(2):
                nc.scalar.activation(q_nat[:, 4 * j: 4 * (j + 1), :],
                                     os_f[:, 4 * j: 4 * (j + 1), :], AF.Tanh)
            nc.vector.tensor_copy(hn[:], hn_f[:])

            # PE-transpose one [128,128] bf16 tile into a transposed layout
            def ptranspose(dst, src):
                tp = psp.tile([P, P], bf16, tag="st", name="tp")
                nc.tensor.transpose(tp[:], src, id_bf[:])
                nc.vector.tensor_copy(dst, tp[:])

            # upfront: the tiles the first loop iterations need
            for t in range(QTPH):
                ptranspose(qT[:, t, :], q_nat[:, t, :])
            for t in range(2):
                ptranspose(ht[:, t, :], hn[:, t, :])
            for j in range(2, 4):
                nc.scalar.activation(q_nat[:, 4 * j: 4 * (j + 1), :],
                                     os_f[:, 4 * j: 4 * (j + 1), :], AF.Tanh)
            # remaining transposes are interleaved into the half-0 loop below
            prefetch = [("h", t) for t in range(2, T)]
            prefetch += [("q", t) for t in range(QTPH, T)]

            # ---- main flash loop ----
            for qh in range(NHALF):
                attnT = pacc.tile([P, QH], f32, tag="acc")   # [h, q_local]
                # per-chunk d accumulators (PSUM)
                dbc = [psd.tile([P, CW], f32, tag="dbc", name=f"dbc{qh}_{i}")
                       for i in range(NC)]
                exprev = [None] * NC
                for kb in range(T):
                    # interleave remaining input transposes (half 0 only):
                    # ht[kb] is consumed at iteration kb, prefetched 2 ahead
                    if qh == 0:
                        for _ in range(2):
                            if prefetch:
                                kind, t = prefetch.pop(0)
                                src = hn if kind == "h" else q_nat
                                dst = ht if kind == "h" else qT
                                ptranspose(dst[:, t, :], src[:, t, :])
                    first = kb == 0
                    last = kb == T - 1
                    for c in range(2):
                        st = psp.tile([P, CW], f32, tag="st")
                        rhs = qT[:, qh * QTPH + 4 * c: qh * QTPH + 4 * (c + 1), :]
                        nc.tensor.matmul(st[:], ht[:, kb, :], rhs,
                                         start=True, stop=True)
                        ex = expool.tile([P, CW], bf16, tag="ex")
                        nc.scalar.activation(ex[:], st[:], AF.Exp)
                        nc.tensor.matmul(attnT[:, CW * c: CW * (c + 1)],
                                         hn[:, kb, :], ex[:],
                                         start=first, stop=last)
                        # d: pair-add exp tiles on DVE (bf16), then
                        # accumulate pair sums via ones-matmul in PSUM
                        if kb % 2 == 0:
                            exprev[c] = ex
                        else:
                            t1 = dtreep.tile([P, CW], bf16, tag="l1")
                            nc.vector.tensor_add(t1[:], exprev[c][:], ex[:])
                            exprev[c] = None
                            nc.tensor.matmul(dbc[c][:], ones_bf[:], t1[:],
                                             start=(kb == 1), stop=last)

                # ---- epilogue for this half ----
                aT_sb = workp.tile([P, QH], f32, tag="atsb")
                nc.vector.tensor_copy(aT_sb[:], attnT[:])
                d_sb = workp.tile([P, QH], f32, tag="dsb")
                for c in range(NC):
                    nc.vector.tensor_copy(d_sb[:, CW * c: CW * (c + 1)], dbc[c][:])
                for t in range(QTPH):
                    dps = psp.tile([P, P], f32, tag="st")
                    nc.tensor.transpose(dps[:], d_sb[:, P * t: P * (t + 1)],
                                        id_f32[:])
                    rc = workp.tile([P, 1], f32, tag="rc")
                    nc.vector.reciprocal(rc[:], dps[:, 0:1])
                    aps = psp.tile([P, P], f32, tag="st")
                    nc.tensor.transpose(aps[:], aT_sb[:, P * t: P * (t + 1)],
                                        id_f32[:])
                    ot = workp.tile([P, P], f32, tag="ot")
                    nc.vector.tensor_scalar_mul(ot[:], aps[:], rc[:])
                    row0 = qh * QH + P * t
                    nc.sync.dma_start(a_d[row0: row0 + P, :], ot[:])

    nc.compile()
    return nc


def _get_nc():
    if "nc" not in _CACHE:
        _CACHE["nc"] = _build()
    return _CACHE["nc"]


def _run(out_state, history, trace=False):
    from concourse.bass_utils import run_bass_kernel_spmd

    nc = _get_nc()
    out_state = np.ascontiguousarray(out_state, dtype=np.float32)
    history = np.ascontiguousarray(history, dtype=np.float32)
    in_maps = [
        {"out_state": out_state[b], "history": history[b]}
        for b in range(N_CORES)
    ]
    res = run_bass_kernel_spmd(nc, in_maps, core_ids=list(range(N_CORES)),
                               trace=trace)
    attn = np.stack([res.results[b]["attn"] for b in range(N_CORES)], axis=0)
    return attn.astype(np.float32), res


def kernel(out_state, history):
    attn, _ = _run(out_state, history)
    return attn


# revision 15
# speedup vs baseline: 1.3792x; 1.0500x over previous
"""Trainium2 Bass kernel for batched tanh-query attention.

Per-batch computation (B=8, one batch per NeuronCore, pure data parallel):
    q = tanh(out_state)            [Q, H]    Q=K=2048, H=128
    S = q @ history.T              [Q, K]
    P = softmax(S, axis=K)
    attn = P @ history             [Q, H]

Flash-style, no HBM intermediates, computed in the transposed orientation
S_T[k, q] so the second matmul needs no transpose of P:
  MM1:  S_T[kb]   = ht[kb].T @ qT          (PE, fp32 PSUM, 512-wide chunks)
  exp:  E[kb]     = exp(S_T[kb])           (ACT, PSUM -> SBUF bf16)
  MM2:  attn_T   += hn[kb].T @ E[kb]       (PE, accumulate over kb)
  d:    pairwise-add tree over E[kb] (DVE bf16) then ones.T @ partials (PE)
  epilogue: PE-transpose attn_T and d back to q-major, multiply by 1/d,
  DMA out. Input operands are bf16; transposed layouts come from the DMA
  xbar transpose.
"""

import sys

for _p in ("/opt/trn_rl_repo", "/opt/trn_rl_repo/concourse"):
    if _p not in sys.path:
        sys.path.insert(0, _p)

import numpy as np

N_CORES = 8
SEQ = 2048
H = 128
P = 128
T = SEQ // P          # 16 seq tiles
NHALF = 2             # queries processed in 2 halves of 1024 (PSUM budget)
QH = SEQ // NHALF     # 1024
QTPH = QH // P        # 8 q-tiles per half
NC = 2                # 512-wide chunks per half
CW = QH // NC         # 512

_CACHE = {}


def _build():
    from concourse import bacc, bass, masks, mybir, tile

    f32 = mybir.dt.float32
    bf16 = mybir.dt.bfloat16
    AF = mybir.ActivationFunctionType

    nc = bacc.Bacc("TRN2", target_bir_lowering=False, debug=False,
                   num_devices=N_CORES)
    os_d = nc.dram_tensor("out_state", (SEQ, H), f32, kind="ExternalInput")
    h_d = nc.dram_tensor("history", (SEQ, H), f32, kind="ExternalInput")
    a_d = nc.dram_tensor("attn", (SEQ, H), f32, kind="ExternalOutput")

    with tile.TileContext(nc) as tc:
        with (
            tc.tile_pool(name="const", bufs=1) as constp,
            tc.tile_pool(name="big", bufs=1) as bigp,
            tc.tile_pool(name="stage", bufs=2) as stagep,
            tc.tile_pool(name="work", bufs=3) as workp,
            tc.tile_pool(name="expool", bufs=6) as expool,
            tc.tile_pool(name="dtree", bufs=4) as dtreep,
            tc.tile_pool(name="ps", bufs=4, space=bass.MemorySpace.PSUM) as psp,
            tc.tile_pool(name="psacc", bufs=1, space=bass.MemorySpace.PSUM) as pacc,
            tc.tile_pool(name="psd", bufs=2, space=bass.MemorySpace.PSUM) as psd,
        ):
            id_f32 = constp.tile([P, P], f32, tag="idf")
            masks.make_identity(nc, id_f32[:])
            id_bf = constp.tile([P, P], bf16, tag="idb")
            masks.make_identity(nc, id_bf[:])
            ones_bf = constp.tile([P, P], bf16, tag="ones")
            nc.vector.memset(ones_bf[:], 1.0)

            # persistent bf16 operands
            hn = bigp.tile([P, T, P], bf16, tag="hn")    # [k_in, t, h] natural
            ht = bigp.tile([P, T, P], bf16, tag="ht")    # [h, t, k_in] transposed
            qT = bigp.tile([P, T, P], bf16, tag="qT")    # [h, t, q_in] transposed

            # ---- load + preprocess (chunked so compute starts early) ----
            os_f = stagep.tile([P, T, H], f32, tag="ldin")
            hn_f = stagep.tile([P, T, H], f32, tag="ldin")
            os_v = os_d[:].rearrange("(t p) h -> p t h", p=P)
            hn_v = h_d[:].rearrange("(t p) h -> p t h", p=P)
            for j in range(4):
                sl = slice(4 * j, 4 * (j + 1))
                nc.sync.dma_start(os_f[:, sl, :], os_v[:, sl, :])
                nc.sync.dma_start(hn_f[:, sl, :], hn_v[:, sl, :])

            q_nat = stagep.tile([P, T, H], bf16, tag="qnat")
            for j in range(2):
                sl = slice(4 * j, 4 * (j + 1))
                nc.scalar.activation(q_nat[:, sl, :], os_f[:, sl, :], AF.Tanh)
                nc.vector.tensor_copy(hn[:, sl, :], hn_f[:, sl, :])

            # PE-transpose one [128,128] bf16 tile into a transposed layout
            def ptranspose(dst, src):
                tp = psp.tile([P, P], bf16, tag="st", name="tp")
                nc.tensor.transpose(tp[:], src, id_bf[:])
                nc.vector.tensor_copy(dst, tp[:])

            # upfront: the tiles the first loop iterations need
            for t in range(QTPH):
                ptranspose(qT[:, t, :], q_nat[:, t, :])
            for t in range(2):
                ptranspose(ht[:, t, :], hn[:, t, :])
            for j in range(2, 4):
                nc.scalar.activation(q_nat[:, 4 * j: 4 * (j + 1), :],
                                     os_f[:, 4 * j: 4 * (j + 1), :], AF.Tanh)
            # remaining transposes are interleaved into the half-0 loop below
            prefetch = [("h", t) for t in range(2, T)]
            prefetch += [("q", t) for t in range(QTPH, T)]

            # ---- epilogue helper: one output q-tile ----
            def emit_epi(qh, t, aT_sb, d_sb):
                dps = psp.tile([P, 1], f32, tag="st", name="dps")
                nc.tensor.transpose(dps[:], d_sb[0:1, P * t: P * (t + 1)],
                                    id_f32[0:1, 0:1])
                rc = workp.tile([P, 1], f32, tag="rc", name="rc")
                nc.vector.reciprocal(rc[:], dps[:])
                aps = psp.tile([P, P], f32, tag="st", name="aps")
                nc.tensor.transpose(aps[:], aT_sb[:, P * t: P * (t + 1)],
                                    id_f32[:])
                ot = workp.tile([P, P], f32, tag="ot", name="ot")
                nc.vector.tensor_scalar_mul(ot[:], aps[:], rc[:])
                row0 = qh * QH + P * t
                nc.sync.dma_start(a_d[row0: row0 + P, :], ot[:])

            epi_pending = []   # half-0 epilogue tiles, drained in half-1 loop

            # ---- main flash loop ----
            for qh in range(NHALF):
                attnT = pacc.tile([P, QH], f32, tag="acc")   # [h, q_local]
                # per-chunk d accumulators (PSUM)
                dbc = [psd.tile([P, CW], f32, tag="dbc", name=f"dbc{qh}_{i}")
                       for i in range(NC)]
                exprev = [None] * NC
                for kb in range(T):
                    if qh == 0:
                        # interleave remaining input transposes: ht[kb] is
                        # consumed at iteration kb, prefetched 2 ahead
                        if kb == 2:
                            for j in range(2, 4):
                                sl = slice(4 * j, 4 * (j + 1))
                                nc.scalar.activation(q_nat[:, sl, :],
                                                     os_f[:, sl, :], AF.Tanh)
                                nc.vector.tensor_copy(hn[:, sl, :],
                                                      hn_f[:, sl, :])
                        for _ in range(2):
                            if prefetch:
                                kind, t = prefetch.pop(0)
                                src = hn if kind == "h" else q_nat
                                dst = ht if kind == "h" else qT
                                ptranspose(dst[:, t, :], src[:, t, :])
                    else:
                        # drain half-0's epilogue tiles
                        if epi_pending:
                            epi_pending.pop(0)()
                    first = kb == 0
                    last = kb == T - 1
                    for c in range(2):
                        st = psp.tile([P, CW], f32, tag="st")
                        rhs = qT[:, qh * QTPH + 4 * c: qh * QTPH + 4 * (c + 1), :]
                        nc.tensor.matmul(st[:], ht[:, kb, :], rhs,
                                         start=True, stop=True)
                        ex = expool.tile([P, CW], bf16, tag="ex")
                        nc.scalar.activation(ex[:], st[:], AF.Exp)
                        nc.tensor.matmul(attnT[:, CW * c: CW * (c + 1)],
                                         hn[:, kb, :], ex[:],
                                         start=first, stop=last)
                        # d: pair-add exp tiles on DVE (bf16), then
                        # accumulate pair sums via ones-matmul in PSUM
                        if kb % 2 == 0:
                            exprev[c] = ex
                        else:
                            t1 = dtreep.tile([P, CW], bf16, tag="l1")
                            nc.vector.tensor_add(t1[:], exprev[c][:], ex[:])
                            exprev[c] = None
                            nc.tensor.matmul(dbc[c][:], ones_bf[:], t1[:],
                                             start=(kb == 1), stop=last)

                # ---- end of half: move accumulators to SBUF ----
                aT_sb = workp.tile([P, QH], f32, tag="atsb", name=f"aT{qh}")
                nc.vector.tensor_copy(aT_sb[:], attnT[:])
                d_sb = workp.tile([P, QH], f32, tag="dsb", name=f"d{qh}")
                for c in range(NC):
                    nc.vector.tensor_copy(d_sb[:, CW * c: CW * (c + 1)],
                                          dbc[c][:])
                if qh == 0:
                    epi_pending.extend(
                        (lambda t=t, a=aT_sb, d=d_sb: emit_epi(0, t, a, d))
                        for t in range(QTPH))
                else:
                    for t in range(QTPH):
                        emit_epi(1, t, aT_sb, d_sb)

    nc.compile()
    return nc


def _get_nc():
    if "nc" not in _CACHE:
        _CACHE["nc"] = _build()
    return _CACHE["nc"]


def _run(out_state, history, trace=False):
    from concourse.bass_utils import run_bass_kernel_spmd

    nc = _get_nc()
    out_state = np.ascontiguousarray(out_state, dtype=np.float32)
    history = np.ascontiguousarray(history, dtype=np.float32)
    in_maps = [
        {"out_state": out_state[b], "history": history[b]}
        for b in range(N_CORES)
    ]
    res = run_bass_kernel_spmd(nc, in_maps, core_ids=list(range(N_CORES)),
                               trace=trace)
    attn = np.stack([res.results[b]["attn"] for b in range(N_CORES)], axis=0)
    return attn.astype(np.float32), res


def kernel(out_state, history):
    attn, _ = _run(out_state, history)
    return attn
